# revision 7
# baseline (speedup 1.0000x reference)
"""Trainium2 Bass kernel for batched multi-head cross-attention.

Problem: qkv (4, 1536, 3072) fp32, packed as 3*(8 heads * 64 ch) along dim 1.
Per (batch, head) item: S = (q*s)^T (k*s)  -> softmax over key axis -> @ v.
bs*heads = 32 independent attention items sharded 4-per-core over 8 cores.

Per-core algorithm (per item, ch=64, T=3072):
  - q,k,v loaded as (64, T) SBUF tiles (channel on partitions). q pre-scaled
    by 1/sqrt(ch) on host (folds both q and k scales).
  - V^T built once per item via PE transpose: 24 blocks (128 s, 64 c), with an
    appended ones-column -> Vt (128, 24*65); the ones-column makes the second
    matmul also produce the softmax denominator row for free.
  - For each 512-wide t-chunk, accumulate over 24 s-blocks of 128:
      MM1  (PE):  S^T block (128 s, 512 t) = k_blk.T @ q_chunk   [f32r]
      EXP  (ACT): W = exp(S^T) for 3 s-blocks at a time (128, 1536) PSUM->SBUF
      MM2  (PE):  acc (65, 512) += Vt_blk.T @ W_blk              [f32r]
    acc rows 0..63 = unnormalized output (c, t), row 64 = sum_s exp = denom.
  - normalize: recip(denom) on DVE, broadcast across 64 partitions with a
    K=1 PE matmul against a ones row, multiply on DVE, DMA out.

Softmax max-subtraction is skipped: S entries are ~N(0,1) (scaled dot of
randn), exp stays in [e-6, e6] -- safely inside fp32 range, and
exp(x)/sum(exp(x)) is algebraically identical to the max-shifted form.
"""

import math
import os
import sys

import numpy as np

for _p in ("/opt/trn_rl_repo", "/opt/pypackages"):
    if os.path.isdir(_p) and _p not in sys.path:
        sys.path.append(_p)

import concourse.bass as bass
import concourse.mybir as mybir
import concourse.tile as tile
from concourse import bacc
from concourse.bass_utils import run_bass_kernel_spmd
from concourse.masks import make_identity

N_CORES = 8
N_HEADS = 8
CH = 64  # head dim
F32 = mybir.dt.float32
F32R = mybir.dt.float32r

TCHUNK = 512  # t columns per psum bank / matmul
SBLK = 128  # s rows per S^T block (psum partitions)
G = 3  # s-blocks per exp() batch: ACT free dim 1536


def build_program(items: int, T: int):
    """Emit the per-core Bass program. All 8 cores run this same program on
    different data (SPMD)."""
    SB = T // SBLK  # number of s blocks
    TC = T // TCHUNK  # number of t chunks
    assert T % TCHUNK == 0 and T % SBLK == 0 and SB % G == 0
    NG = SB // G
    CW = CH + 1  # Vt block width (64 cols of v^T + ones column)

    nc = bacc.Bacc(
        "TRN2", target_bir_lowering=False, debug=False, num_devices=N_CORES
    )
    # q/k feed f32r matmuls; walrus requires every producer reaching an f32r
    # matmul to carry the f32r dtype (pre-rounded), so declare them f32r all
    # the way from DRAM (bit-identical layout to f32 on the host side).
    qd = nc.dram_tensor("q", [items, CH, T], F32R, kind="ExternalInput")
    kd = nc.dram_tensor("k", [items, CH, T], F32R, kind="ExternalInput")
    vd = nc.dram_tensor("v", [items, CH, T], F32, kind="ExternalInput")
    od = nc.dram_tensor("out", [items, CH, T], F32, kind="ExternalOutput")

    EXP = mybir.ActivationFunctionType.Exp

    with tile.TileContext(nc) as tc:
        with (
            tc.tile_pool(name="const", bufs=1) as cpool,
            tc.tile_pool(name="qkv", bufs=2) as qkpool,
            tc.tile_pool(name="vt", bufs=2) as vtpool,
            tc.tile_pool(name="w", bufs=3) as wpool,
            tc.tile_pool(name="osb", bufs=3) as opool,
            tc.tile_pool(name="rc", bufs=2) as rcpool,
            # PSUM budget (8 banks): s-tiles 2x3 + acc 1 + misc 1
            tc.tile_pool(name="spsum", bufs=2, space="PSUM") as spool,
            tc.tile_pool(name="accpsum", bufs=1, space="PSUM") as accpool,
            tc.tile_pool(name="miscpsum", bufs=1, space="PSUM") as mpool,
        ):
            ident = cpool.tile([CH, CH], F32)
            make_identity(nc, ident[:])
            # memset can't write f32r; memset f32 staging then DVE-convert.
            ones_f32 = cpool.tile([1, CH], F32)
            nc.vector.memset(ones_f32[:], 1.0)
            ones_row = cpool.tile([1, CH], F32R)
            nc.vector.tensor_copy(ones_row[:], ones_f32[:])
            ones_blk = cpool.tile([SBLK, SB], F32)
            nc.vector.memset(ones_blk[:], 1.0)

            for it in range(items):
                q_sb = qkpool.tile([CH, T], F32R, tag="q")
                nc.sync.dma_start(q_sb[:], qd[it])
                k_sb = qkpool.tile([CH, T], F32R, tag="k")
                nc.sync.dma_start(k_sb[:], kd[it])
                v_sb = qkpool.tile([CH, T], F32, tag="v")
                nc.sync.dma_start(v_sb[:], vd[it])

                # Vt: 24 transposed v-blocks, each (128 s, 64 c) + ones col.
                vt = vtpool.tile([SBLK, SB * CW], F32R, tag="vt")
                for s in range(SB):
                    tp = mpool.tile([SBLK, CH], F32, tag="misc")
                    nc.tensor.transpose(tp[:], v_sb[:, bass.ts(s, SBLK)], ident[:])
                    nc.vector.tensor_copy(vt[:, s * CW : s * CW + CH], tp[:])
                ones_cols = vt[:].rearrange("p (s c) -> p s c", c=CW)[:, :, CH : CH + 1]
                nc.vector.tensor_copy(
                    ones_cols, ones_blk[:].rearrange("p (s o) -> p s o", o=1)
                )

                for tci in range(TC):
                    acc = accpool.tile([CW, TCHUNK], F32, tag="acc")
                    for g in range(NG):
                        st = spool.tile([SBLK, TCHUNK * G], F32, tag="s")
                        for j in range(G):
                            sidx = g * G + j
                            nc.tensor.matmul(
                                st[:, TCHUNK * j : TCHUNK * (j + 1)],
                                lhsT=k_sb[:, bass.ts(sidx, SBLK)],
                                rhs=q_sb[:, bass.ts(tci, TCHUNK)],
                                start=True,
                                stop=True,
                            )
                        w = wpool.tile([SBLK, TCHUNK * G], F32R, tag="w")
                        nc.scalar.activation(w[:], st[:], EXP)
                        for j in range(G):
                            sidx = g * G + j
                            nc.tensor.matmul(
                                acc[:],
                                lhsT=vt[:, sidx * CW : (sidx + 1) * CW],
                                rhs=w[:, TCHUNK * j : TCHUNK * (j + 1)],
                                start=(sidx == 0),
                                stop=(sidx == SB - 1),
                                skip_group_check=True,
                            )
                    rc = rcpool.tile([1, TCHUNK], F32R, tag="rc")
                    with nc.allow_low_precision("softmax reciprocal rounds to f32r"):
                        nc.vector.reciprocal(rc[:], acc[CH : CH + 1, :])
                    bc = mpool.tile([CH, TCHUNK], F32, tag="misc")
                    nc.tensor.matmul(
                        bc[:],
                        lhsT=ones_row[:],
                        rhs=rc[:],
                        start=True,
                        stop=True,
                    )
                    bcs = opool.tile([CH, TCHUNK], F32, tag="bcs")
                    nc.vector.tensor_copy(bcs[:], bc[:])
                    osb = opool.tile([CH, TCHUNK], F32, tag="osb")
                    nc.vector.tensor_mul(osb[:], acc[0:CH, :], bcs[:])
                    nc.sync.dma_start(od[it][:, bass.ts(tci, TCHUNK)], osb[:])

    nc.compile()
    return nc


_CACHE: dict = {}


def _get_program(items: int, T: int):
    key = (items, T)
    if key not in _CACHE:
        _CACHE[key] = build_program(items, T)
    return _CACHE[key]


def _host_split(qkv: np.ndarray):
    """Split packed qkv into per-item q (pre-scaled), k, v of shape
    (bs*heads, ch, T)."""
    bs, width, T = qkv.shape
    ch = width // (3 * N_HEADS)
    q = qkv[:, : width // 3]
    k = qkv[:, width // 3 : 2 * (width // 3)]
    v = qkv[:, 2 * (width // 3) :]
    scale2 = 1.0 / math.sqrt(ch)  # (ch**-0.25)**2 folded into q
    qh = (q * np.float32(scale2)).reshape(bs * N_HEADS, ch, T)
    kh = k.reshape(bs * N_HEADS, ch, T)
    vh = v.reshape(bs * N_HEADS, ch, T)
    return qh, kh, vh


def kernel(qkv, l):
    qkv = np.asarray(qkv, dtype=np.float32)
    l = int(l)
    bs, width, T = qkv.shape
    ch = width // (3 * N_HEADS)
    assert ch == CH, f"unexpected head dim {ch}"

    qh, kh, vh = _host_split(qkv)
    n_items = bs * N_HEADS
    ipc = n_items // N_CORES  # items per core

    nc = _get_program(ipc, T)
    in_maps = [
        {
            "q": np.ascontiguousarray(qh[c * ipc : (c + 1) * ipc]),
            "k": np.ascontiguousarray(kh[c * ipc : (c + 1) * ipc]),
            "v": np.ascontiguousarray(vh[c * ipc : (c + 1) * ipc]),
        }
        for c in range(N_CORES)
    ]
    res = run_bass_kernel_spmd(nc, in_maps, list(range(N_CORES)))
    agg = np.concatenate([res.results[c]["out"] for c in range(N_CORES)], axis=0)
    agg = agg.reshape(bs, N_HEADS * ch, T)
    return (agg[:, :, :l], agg[:, :, l : 2 * l], agg[:, :, 2 * l :])


# revision 18
# speedup vs baseline: 5041.9015x; 5041.9015x over previous
"""Trainium2 Bass kernel for batched multi-head cross-attention.

Problem: qkv (4, 1536, 3072) fp32, packed as 3*(8 heads * 64 ch) along dim 1.
Per (batch, head) item: S = (q*s)^T (k*s)  -> softmax over key axis -> @ v.
bs*heads = 32 independent attention items sharded 4-per-core over 8 cores.

Per-core algorithm (per item, ch=64, T=3072):
  - q,k,v loaded as (64, T) SBUF tiles (channel on partitions). q pre-scaled
    by 1/sqrt(ch) on host (folds both q and k scales).
  - V^T built once per item via PE transpose: 24 blocks (128 s, 64 c), with an
    appended ones-column -> Vt (128, 24*65); the ones-column makes the second
    matmul also produce the softmax denominator row for free.
  - For each 512-wide t-chunk, accumulate over 24 s-blocks of 128:
      MM1  (PE):  S^T block (128 s, 512 t) = k_blk.T @ q_chunk   [fp16]
      EXP  (ACT): W = exp(S^T) for 3 s-blocks at a time (128, 1536) PSUM->SBUF
      MM2  (PE):  acc (65, 512) += Vt_blk.T @ W_blk              [fp16]
    acc rows 0..63 = unnormalized output (c, t), row 64 = sum_s exp = denom.
  - normalize: recip(denom) on DVE, broadcast across 64 partitions with a
    K=1 PE matmul against a ones row, multiply on DVE, DMA out.

Softmax max-subtraction is skipped: S entries are ~N(0,1) (scaled dot of
randn), exp stays in [e-6, e6] -- safely inside fp32 range, and
exp(x)/sum(exp(x)) is algebraically identical to the max-shifted form.
"""

import math
import os
import sys

import numpy as np

for _p in ("/opt/trn_rl_repo", "/opt/pypackages"):
    if os.path.isdir(_p) and _p not in sys.path:
        sys.path.append(_p)

import concourse.bass as bass
import concourse.mybir as mybir
import concourse.tile as tile
from concourse import bacc
from concourse.bass_utils import run_bass_kernel_spmd
from concourse.masks import make_identity

N_CORES = 8
N_HEADS = 8
CH = 64  # head dim
F32 = mybir.dt.float32
F32R = mybir.dt.float32r
F16 = mybir.dt.float16

# dtype of all matmul operands (q, k, Vt, W, ones, recip). fp16 streams at
# 1 col/cycle on the PE (4-byte f32r measured ~4x slower) and keeps ~5e-4
# relative precision, far better than bf16.
MM_DT = F16
MM_NP = np.float16

TCHUNK = 512  # t columns per psum bank / matmul
SBLK = 128  # s rows per S^T block (psum partitions)
G = 3  # s-blocks per exp() batch: ACT free dim 1536


def build_program(items: int, T: int, repeat: int = 1, stages: str = "full"):
    """Emit the per-core Bass program. All 8 cores run this same program on
    different data (SPMD). repeat>1 wraps the body in a hardware loop (used
    only for timing: device time scales with repeat, host overhead doesn't).
    stages: 'mm1' | 'mm1exp' | 'mm1expmm2' | 'full' — timing ablations."""
    do_exp = stages != "mm1"
    do_mm2 = stages in ("mm1expmm2", "full")
    do_norm = stages == "full"
    SB = T // SBLK  # number of s blocks
    TC = T // TCHUNK  # number of t chunks
    assert T % TCHUNK == 0 and T % SBLK == 0 and SB % G == 0
    NG = SB // G
    CW = CH + 1  # Vt block width (64 cols of v^T + ones column)

    nc = bacc.Bacc(
        "TRN2", target_bir_lowering=False, debug=False, num_devices=N_CORES
    )
    # q/k are sent from the host already converted to the matmul dtype
    # (halves the input DMA traffic as well).
    qd = nc.dram_tensor("q", [items, CH, T], MM_DT, kind="ExternalInput")
    kd = nc.dram_tensor("k", [items, CH, T], MM_DT, kind="ExternalInput")
    vd = nc.dram_tensor("v", [items, CH, T], F32, kind="ExternalInput")
    od = nc.dram_tensor("out", [items, CH, T], F32, kind="ExternalOutput")

    EXP = mybir.ActivationFunctionType.Exp

    with tile.TileContext(nc) as tc:
        with (
            tc.tile_pool(name="const", bufs=1) as cpool,
            tc.tile_pool(name="qkv", bufs=2) as qkpool,
            tc.tile_pool(name="vt", bufs=2) as vtpool,
            tc.tile_pool(name="w", bufs=3) as wpool,
            tc.tile_pool(name="osb", bufs=3) as opool,
            tc.tile_pool(name="rc", bufs=2) as rcpool,
            # PSUM budget (8 banks): s-tiles 2x3 + acc 1 + misc 1
            tc.tile_pool(name="spsum", bufs=2, space="PSUM") as spool,
            tc.tile_pool(name="accpsum", bufs=1, space="PSUM") as accpool,
            tc.tile_pool(name="miscpsum", bufs=1, space="PSUM") as mpool,
        ):
            ident = cpool.tile([CH, CH], F32)
            make_identity(nc, ident[:])
            # memset can't write f32r; go through f32 staging + DVE convert
            # (also fine for fp16).
            ones_f32 = cpool.tile([1, CH], F32)
            nc.vector.memset(ones_f32[:], 1.0)
            ones_row = cpool.tile([1, CH], MM_DT)
            nc.vector.tensor_copy(ones_row[:], ones_f32[:])
            ones_blk = cpool.tile([SBLK, SB], F32)
            nc.vector.memset(ones_blk[:], 1.0)

            def body():
                for it in range(items):
                    emit_item(it)
                if not do_norm:
                    # ablation builds: keep the output tensor written
                    nc.sync.dma_start(od[0][:, 0:SB], ones_blk[0:CH, :])

            def emit_item(it):
                q_sb = qkpool.tile([CH, T], MM_DT, tag="q")
                nc.sync.dma_start(q_sb[:], qd[it])
                k_sb = qkpool.tile([CH, T], MM_DT, tag="k")
                nc.sync.dma_start(k_sb[:], kd[it])
                v_sb = qkpool.tile([CH, T], F32, tag="v")
                nc.sync.dma_start(v_sb[:], vd[it])

                # Vt: 24 transposed v-blocks, each (128 s, 64 c) + ones col.
                vt = vtpool.tile([SBLK, SB * CW], MM_DT, tag="vt")
                if do_mm2:
                    for s in range(SB):
                        tp = mpool.tile([SBLK, CH], F32, tag="misc")
                        nc.tensor.transpose(tp[:], v_sb[:, bass.ts(s, SBLK)], ident[:])
                        nc.vector.tensor_copy(vt[:, s * CW : s * CW + CH], tp[:])
                    ones_cols = vt[:].rearrange("p (s c) -> p s c", c=CW)[
                        :, :, CH : CH + 1
                    ]
                    nc.vector.tensor_copy(
                        ones_cols, ones_blk[:].rearrange("p (s o) -> p s o", o=1)
                    )

                for tci in range(TC):
                    acc = accpool.tile([CW, TCHUNK], F32, tag="acc")
                    for g in range(NG):
                        st = spool.tile([SBLK, TCHUNK * G], F32, tag="s")
                        for j in range(G):
                            sidx = g * G + j
                            nc.tensor.matmul(
                                st[:, TCHUNK * j : TCHUNK * (j + 1)],
                                lhsT=k_sb[:, bass.ts(sidx, SBLK)],
                                rhs=q_sb[:, bass.ts(tci, TCHUNK)],
                                start=True,
                                stop=True,
                            )
                        w = wpool.tile([SBLK, TCHUNK * G], MM_DT, tag="w")
                        if do_exp:
                            nc.scalar.activation(w[:], st[:], EXP)
                        if do_mm2:
                            for j in range(G):
                                sidx = g * G + j
                                nc.tensor.matmul(
                                    acc[:],
                                    lhsT=vt[:, sidx * CW : (sidx + 1) * CW],
                                    rhs=w[:, TCHUNK * j : TCHUNK * (j + 1)],
                                    start=(sidx == 0),
                                    stop=(sidx == SB - 1),
                                    skip_group_check=True,
                                )
                    if not do_norm:
                        continue
                    rc = rcpool.tile([1, TCHUNK], MM_DT, tag="rc")
                    with nc.allow_low_precision("softmax reciprocal rounds to f32r"):
                        nc.vector.reciprocal(rc[:], acc[CH : CH + 1, :])
                    bc = mpool.tile([CH, TCHUNK], F32, tag="misc")
                    nc.tensor.matmul(
                        bc[:],
                        lhsT=ones_row[:],
                        rhs=rc[:],
                        start=True,
                        stop=True,
                    )
                    bcs = opool.tile([CH, TCHUNK], F32, tag="bcs")
                    nc.vector.tensor_copy(bcs[:], bc[:])
                    osb = opool.tile([CH, TCHUNK], F32, tag="osb")
                    nc.vector.tensor_mul(osb[:], acc[0:CH, :], bcs[:])
                    nc.sync.dma_start(od[it][:, bass.ts(tci, TCHUNK)], osb[:])

            if repeat > 1:
                with tc.For_i(0, repeat, 1):
                    body()
            else:
                body()

    nc.compile()
    return nc


_CACHE: dict = {}


def _get_program(items: int, T: int):
    key = (items, T)
    if key not in _CACHE:
        _CACHE[key] = build_program(items, T)
    return _CACHE[key]


def _host_split(qkv: np.ndarray):
    """Split packed qkv into per-item q (pre-scaled), k, v of shape
    (bs*heads, ch, T)."""
    bs, width, T = qkv.shape
    ch = width // (3 * N_HEADS)
    q = qkv[:, : width // 3]
    k = qkv[:, width // 3 : 2 * (width // 3)]
    v = qkv[:, 2 * (width // 3) :]
    scale2 = 1.0 / math.sqrt(ch)  # (ch**-0.25)**2 folded into q
    qh = (q * np.float32(scale2)).reshape(bs * N_HEADS, ch, T).astype(MM_NP)
    kh = k.reshape(bs * N_HEADS, ch, T).astype(MM_NP)
    vh = v.reshape(bs * N_HEADS, ch, T)
    return qh, kh, vh


def kernel(qkv, l):
    qkv = np.asarray(qkv, dtype=np.float32)
    l = int(l)
    bs, width, T = qkv.shape
    ch = width // (3 * N_HEADS)
    assert ch == CH, f"unexpected head dim {ch}"

    qh, kh, vh = _host_split(qkv)
    n_items = bs * N_HEADS
    ipc = n_items // N_CORES  # items per core

    nc = _get_program(ipc, T)
    in_maps = [
        {
            "q": np.ascontiguousarray(qh[c * ipc : (c + 1) * ipc]),
            "k": np.ascontiguousarray(kh[c * ipc : (c + 1) * ipc]),
            "v": np.ascontiguousarray(vh[c * ipc : (c + 1) * ipc]),
        }
        for c in range(N_CORES)
    ]
    res = run_bass_kernel_spmd(nc, in_maps, list(range(N_CORES)))
    agg = np.concatenate([res.results[c]["out"] for c in range(N_CORES)], axis=0)
    agg = agg.reshape(bs, N_HEADS * ch, T)
    return (agg[:, :, :l], agg[:, :, l : 2 * l], agg[:, :, 2 * l :])
